# revision 13
# baseline (speedup 1.0000x reference)
"""Trainium2 Bass kernel for nn_MultiLevelGCN (3-layer dense GCN + MLP head).

Computation (reference):
    x = x0
    for l in range(3): x = relu((A @ x) @ W_l^T + b_l)
    h = x[nodes_idx]
    encode = relu(h @ w1^T + b1)
    out = softmax(encode @ w2^T + b2)

Sharding: 1-D row partition of A over 8 cores (2048 rows each). Each core
computes its slice of A @ x with the full x; between layers the x slices are
exchanged with chunked AllGathers (4 chunks of 512 rows per layer) so comm
overlaps the next layer's compute. The A operand is passed pre-transposed
and pre-cast to bf16 (A^T column slice, [16384, 2048] per core) so both
matmul operands have the contraction (node) dim on SBUF partitions and the
TensorE runs at 1 cycle/row with the stationary load pipelined (bf16 emits a
separate LDWEIGHTS that the PE queue pulls ahead; the fused 4-byte f32r load
is not pipelined and measures ~1.8x slower). PSUM accumulates fp32. The
small per-layer weight matmul runs f32r so z keeps near-fp32 precision.

Per layer, per core:
    z.T[f, i] = sum_j x[j, f] * A^T[j, i]   (x block stationary, A^T moving)
    xn.T[g, i] = relu(sum_f W^T[f, g] * z.T[f, i] + b[g])
    transpose xn.T -> xn (node-major bf16), store, chunked AllGather.
Head: indirect-DMA row gather of the all-gathered x3 by (permuted)
nodes_idx, MLP + softmax on 1024 rows per core.

DMA: A-stream loads are 512 KB ([512 rows, 512 cols] bf16) alternating
between the two HWDGE rings (sync=qSPDynamicHW, scalar=qActDynamicHW);
x loads are one 1 MB DMA per all-gather chunk. Measured: one ring sustains
~244 GB/s, two rings ~330 GB/s.
"""

import sys

if "/opt/trn_rl_repo" not in sys.path:
    sys.path.insert(0, "/opt/trn_rl_repo")

import ml_dtypes
import numpy as np

import concourse.bass as bass
import concourse.mybir as mybir
import concourse.tile as tile
from concourse import bacc
from concourse.masks import make_identity

N = 16384      # nodes
F = 256        # feature dim
L = 3          # gcn layers
MLP_H = 128    # mlp hidden
N_CLS = 16     # classes
N_IDX = 8192   # labeled nodes
C = 8          # cores
S = N // C     # rows per core = 2048
Q = 4          # all-gather chunks per layer
CH = S // Q    # rows per chunk = 512
P = 128        # partitions
NB = N // P    # j-blocks = 128
IC = 4         # output column chunks per core (512 each)
ICW = S // IC  # 512
NG = N // CH   # 512-row groups of the contraction dim = 32

F32 = mybir.dt.float32
F32R = mybir.dt.float32r
BF16 = mybir.dt.bfloat16
I32 = mybir.dt.int32
NPBF16 = ml_dtypes.bfloat16

AT_BUFS = 8
X_BUFS = 5
TRACE = False
LAST_EXEC_NS = None
LAST_RESULTS = None

_CACHED = None


def _build():
    nc = bacc.Bacc(trn_type="TRN2", target_bir_lowering=False, debug=False,
                   num_devices=C)

    # ---- external I/O (per core) ----
    at_d = nc.dram_tensor("at", [N, S], BF16, kind="ExternalInput")   # A^T slice, bf16
    x0_d = nc.dram_tensor("x0b", [N, F], BF16, kind="ExternalInput")  # x0, bf16
    wt_d = nc.dram_tensor("wt", [L, F, F], F32, kind="ExternalInput")  # W^T per layer [f_in, g_out]
    bias_d = nc.dram_tensor("bias", [L, F, 1], F32, kind="ExternalInput")
    w1t_d = nc.dram_tensor("w1t", [F, MLP_H], BF16, kind="ExternalInput")
    b1_d = nc.dram_tensor("b1", [MLP_H, 1], F32, kind="ExternalInput")
    w2t_d = nc.dram_tensor("w2t", [MLP_H, N_CLS], F32, kind="ExternalInput")
    b2_d = nc.dram_tensor("b2", [1, N_CLS], F32, kind="ExternalInput")
    idx_d = nc.dram_tensor("idxp", [N_IDX // C, 1], I32, kind="ExternalInput")
    enc_d = nc.dram_tensor("enc", [N_IDX // C, MLP_H], F32, kind="ExternalOutput")
    out_d = nc.dram_tensor("out", [N_IDX // C, N_CLS], F32, kind="ExternalOutput")

    # ---- internal DRAM (bf16 activations) ----
    xsl = [[nc.dram_tensor(f"xsl_{l}_{q}", [CH, F], BF16) for q in range(Q)]
           for l in range(L)]
    xg = {l: [nc.dram_tensor(f"xg_{l}_{q}", [C * CH, F], BF16, addr_space="Shared")
              for q in range(Q)]
          for l in (1, 2)}
    # Final gathered x3: one tensor so indirect DMA gathers from offset 0.
    # Row layout: q*4096 + c*512 + r  for global node j = c*2048 + q*512 + r.
    xg3_d = nc.dram_tensor("xg3", [N, F], BF16, addr_space="Shared")

    rg = [list(range(C))]

    dma_engines = [nc.sync, nc.scalar]  # the two HWDGE rings

    with tile.TileContext(nc) as tc:
        with (
            tc.tile_pool(name="xres", bufs=X_BUFS) as x_pool,
            tc.tile_pool(name="at", bufs=AT_BUFS) as at_pool,
            tc.tile_pool(name="zt", bufs=4) as zt_pool,
            tc.tile_pool(name="xnt", bufs=4) as xnt_pool,
            tc.tile_pool(name="xn", bufs=4) as xn_pool,
            tc.tile_pool(name="wconst", bufs=1) as w_pool,
            tc.tile_pool(name="head", bufs=4) as h_pool,
            tc.tile_pool(name="headc", bufs=1) as hc_pool,
            tc.tile_pool(name="sm", bufs=8) as sm_pool,
            tc.tile_pool(name="pz", bufs=4, space="PSUM") as pz_pool,
            tc.tile_pool(name="px", bufs=2, space="PSUM") as px_pool,
            tc.tile_pool(name="pt", bufs=2, space="PSUM") as pt_pool,
        ):
            # ---- constants ----
            ident_b = w_pool.tile([P, P], BF16, tag="identb")
            make_identity(nc, ident_b[:])
            ident_f = w_pool.tile([P, P], F32, tag="identf")
            make_identity(nc, ident_f[:])
            wt_sb = w_pool.tile([P, L * 2, F], F32R, tag="wt")
            for l in range(L):
                for fb in range(2):
                    nc.sync.dma_start(
                        out=wt_sb[:, l * 2 + fb, :],
                        in_=wt_d[l, fb * P:(fb + 1) * P, :].bitcast(F32R))
            bias_sb = w_pool.tile([P, L * 2], F32, tag="bias")
            for l in range(L):
                for gb in range(2):
                    nc.sync.dma_start(
                        out=bias_sb[:, l * 2 + gb:l * 2 + gb + 1],
                        in_=bias_d[l, gb * P:(gb + 1) * P, :])
            w1t_sb = w_pool.tile([P, 2, MLP_H], BF16, tag="w1t")
            for fb in range(2):
                nc.sync.dma_start(out=w1t_sb[:, fb, :],
                                  in_=w1t_d[fb * P:(fb + 1) * P, :])
            b1_sb = w_pool.tile([P, 1], F32, tag="b1")
            nc.sync.dma_start(out=b1_sb[:, :], in_=b1_d[:, :])
            w2t_sb = w_pool.tile([P, N_CLS], F32, tag="w2t")
            nc.sync.dma_start(out=w2t_sb[:, :], in_=w2t_d[:, :])
            b2_sb = w_pool.tile([1, N_CLS], F32, tag="b2")
            nc.sync.dma_start(out=b2_sb[:, :], in_=b2_d[:, :])
            ones_sb = w_pool.tile([1, P], F32, tag="ones")
            nc.gpsimd.memset(ones_sb[:, :], 1.0)

            # ---- GCN layers ----
            # Contraction rows are processed in 512-row groups. Group g covers
            # global nodes [g*512, (g+1)*512). For l>=1 availability order is
            # chunk-q major (gathered chunk q lands first); layer 0 is natural.
            for l in range(L):
                if l == 0:
                    g_list = list(range(NG))
                else:
                    g_list = [c * Q + q for q in range(Q) for c in range(C)]

                # x resident tiles: one [128, 32, 256] bf16 tile per source
                # region (layer 0: x0 quarters; l>=1: gathered chunk q).
                xt = {}
                for i, q in enumerate(range(Q)):
                    t = x_pool.tile([P, (N // Q) // P, F], BF16, tag="x", name="x")
                    if l == 0:
                        src = x0_d[q * (N // Q):(q + 1) * (N // Q), :]
                    else:
                        src = xg[l][q][:, :]
                    # DRAM rows (s*128 + p, f) -> SBUF (p, s, f)
                    dma_engines[i % 2].dma_start(
                        out=t[:], in_=src.rearrange("(s p) f -> p s f", p=P))
                    xt[q] = t

                def x_slot(j0):
                    # (tile, slot) holding global nodes [j0, j0+128)
                    if l == 0:
                        return xt[j0 // (N // Q)], (j0 % (N // Q)) // P
                    c, rem = divmod(j0, S)
                    q, r = divmod(rem, CH)
                    return xt[q], c * (CH // P) + r // P

                for ic in range(IC):
                    pz = [pz_pool.tile([P, ICW], F32, tag="pz", name="pz")
                          for _ in range(2)]
                    for gi, g in enumerate(g_list):
                        at_t = at_pool.tile([P, CH // P, ICW], BF16, tag="at",
                                            name="at")
                        dma_engines[gi % 2].dma_start(
                            out=at_t[:],
                            in_=at_d[g * CH:(g + 1) * CH,
                                     ic * ICW:(ic + 1) * ICW]
                            .rearrange("(r p) c -> p r c", p=P))
                        for rb in range(CH // P):
                            xtile, slot = x_slot(g * CH + rb * P)
                            for fb in range(2):
                                nc.tensor.matmul(
                                    out=pz[fb][:],
                                    lhsT=xtile[:, slot, fb * P:(fb + 1) * P],
                                    rhs=at_t[:, rb, :],
                                    start=(gi == 0 and rb == 0),
                                    stop=(gi == NG - 1 and rb == CH // P - 1))

                    zt = []
                    for fb in range(2):
                        z = zt_pool.tile([P, ICW], F32R, tag="zt")
                        nc.vector.tensor_copy(out=z[:], in_=pz[fb][:])
                        zt.append(z)

                    xnt = []
                    for gb in range(2):
                        px = px_pool.tile([P, ICW], F32, tag="px")
                        for fb in range(2):
                            nc.tensor.matmul(
                                out=px[:],
                                lhsT=wt_sb[:, l * 2 + fb, gb * P:(gb + 1) * P],
                                rhs=zt[fb][:],
                                start=(fb == 0), stop=(fb == 1))
                        xt_out = xnt_pool.tile([P, ICW], BF16, tag="xnt")
                        nc.scalar.activation(
                            out=xt_out[:], in_=px[:],
                            func=mybir.ActivationFunctionType.Relu,
                            bias=bias_sb[:, l * 2 + gb:l * 2 + gb + 1])
                        xnt.append(xt_out)

                    for isub in range(ICW // P):
                        xn = xn_pool.tile([P, F], BF16, tag="xn")
                        for gb in range(2):
                            ptt = pt_pool.tile([P, P], BF16, tag="pt")
                            nc.tensor.transpose(
                                out=ptt[:], in_=xnt[gb][:, isub * P:(isub + 1) * P],
                                identity=ident_b[:])
                            nc.vector.tensor_copy(
                                out=xn[:, gb * P:(gb + 1) * P], in_=ptt[:])
                        nc.sync.dma_start(
                            out=xsl[l][ic][isub * P:(isub + 1) * P, :], in_=xn[:])

                    outs = (xg[l + 1][ic][:, :] if l < L - 1
                            else xg3_d[ic * C * CH:(ic + 1) * C * CH, :])
                    nc.gpsimd.collective_compute(
                        "AllGather", mybir.AluOpType.bypass,
                        replica_groups=rg,
                        ins=[xsl[l][ic][:, :]],
                        outs=[outs])

            # ---- head: gather + MLP + softmax ----
            n_ch = (N_IDX // C) // P  # 8 chunks of 128 indices
            ht = [hc_pool.tile([P, n_ch * P], BF16, tag=f"ht{fb}", name=f"ht{fb}")
                  for fb in range(2)]
            for ch in range(n_ch):
                idx_t = h_pool.tile([P, 1], I32, tag="idx")
                nc.sync.dma_start(out=idx_t[:], in_=idx_d[ch * P:(ch + 1) * P, :])
                h = h_pool.tile([P, F], BF16, tag="h")
                nc.gpsimd.indirect_dma_start(
                    out=h[:], out_offset=None,
                    in_=xg3_d[:, :],
                    in_offset=bass.IndirectOffsetOnAxis(ap=idx_t[:, :1], axis=0))
                for fb in range(2):
                    ptt = pt_pool.tile([P, P], BF16, tag="pt")
                    nc.tensor.transpose(out=ptt[:], in_=h[:, fb * P:(fb + 1) * P],
                                        identity=ident_b[:])
                    nc.vector.tensor_copy(out=ht[fb][:, ch * P:(ch + 1) * P],
                                          in_=ptt[:])

            et = hc_pool.tile([P, n_ch * P], F32, tag="et")  # encode.T [m, i2]
            for i2c in range(2):
                pe = px_pool.tile([P, ICW], F32, tag="px")
                for fb in range(2):
                    nc.tensor.matmul(
                        out=pe[:],
                        lhsT=w1t_sb[:, fb, :],
                        rhs=ht[fb][:, i2c * ICW:(i2c + 1) * ICW],
                        start=(fb == 0), stop=(fb == 1))
                nc.scalar.activation(
                    out=et[:, i2c * ICW:(i2c + 1) * ICW], in_=pe[:],
                    func=mybir.ActivationFunctionType.Relu,
                    bias=b1_sb[:, 0:1])

            for ch in range(n_ch):
                ptt = pt_pool.tile([P, P], F32, tag="pt", name="ptf")
                nc.tensor.transpose(out=ptt[:], in_=et[:, ch * P:(ch + 1) * P],
                                    identity=ident_f[:])
                enc_t = h_pool.tile([P, MLP_H], F32, tag="enc")
                nc.vector.tensor_copy(out=enc_t[:], in_=ptt[:])
                nc.sync.dma_start(out=enc_d[ch * P:(ch + 1) * P, :], in_=enc_t[:])

                pl = pt_pool.tile([P, N_CLS], F32, tag="pt", name="ptf")
                nc.tensor.matmul(out=pl[:], lhsT=ones_sb[:, :], rhs=b2_sb[:, :],
                                 start=True, stop=False, skip_group_check=True)
                nc.tensor.matmul(out=pl[:], lhsT=et[:, ch * P:(ch + 1) * P],
                                 rhs=w2t_sb[:, :],
                                 start=False, stop=True, skip_group_check=True)
                nmax = sm_pool.tile([P, 1], F32, tag="nmax")
                nc.vector.tensor_reduce(out=nmax[:], in_=pl[:],
                                        axis=mybir.AxisListType.X,
                                        op=mybir.AluOpType.max, negate=True)
                ex = sm_pool.tile([P, N_CLS], F32, tag="ex")
                ssum = sm_pool.tile([P, 1], F32, tag="ssum")
                nc.scalar.activation(out=ex[:], in_=pl[:],
                                     func=mybir.ActivationFunctionType.Exp,
                                     bias=nmax[:, 0:1],
                                     accum_out=ssum[:, 0:1])
                rs = sm_pool.tile([P, 1], F32, tag="rs")
                nc.vector.reciprocal(out=rs[:], in_=ssum[:])
                ot = sm_pool.tile([P, N_CLS], F32, tag="ot")
                nc.scalar.activation(out=ot[:], in_=ex[:],
                                     func=mybir.ActivationFunctionType.Copy,
                                     scale=rs[:, 0:1])
                nc.sync.dma_start(out=out_d[ch * P:(ch + 1) * P, :], in_=ot[:])

    nc.compile()
    return nc


def _prep_inputs(A, x0, gcn_w, gcn_b, mlp_w1, mlp_b1, mlp_w2, mlp_b2, nodes_idx):
    A = np.asarray(A, dtype=np.float32)
    Ab = A.astype(NPBF16)
    x0b = np.ascontiguousarray(np.asarray(x0, dtype=np.float32)).astype(NPBF16)
    wt = np.ascontiguousarray(np.asarray(gcn_w, np.float32).transpose(0, 2, 1))
    bias = np.ascontiguousarray(np.asarray(gcn_b, np.float32).reshape(L, F, 1))
    w1t = np.ascontiguousarray(np.asarray(mlp_w1, np.float32).T).astype(NPBF16)
    b1 = np.ascontiguousarray(np.asarray(mlp_b1, np.float32).reshape(MLP_H, 1))
    w2t = np.ascontiguousarray(np.asarray(mlp_w2, np.float32).T)
    b2 = np.ascontiguousarray(np.asarray(mlp_b2, np.float32).reshape(1, N_CLS))
    idx = np.asarray(nodes_idx).astype(np.int64)
    # permute indices into the xg3 row layout: q*4096 + c*512 + r
    c = idx // S
    q = (idx % S) // CH
    r = idx % CH
    idxp = (q * (C * CH) + c * CH + r).astype(np.int32)

    in_maps = []
    for cc in range(C):
        at_c = np.ascontiguousarray(Ab[cc * S:(cc + 1) * S, :].T)
        in_maps.append({
            "at": at_c,
            "x0b": x0b,
            "wt": wt,
            "bias": bias,
            "w1t": w1t,
            "b1": b1,
            "w2t": w2t,
            "b2": b2,
            "idxp": idxp[cc * (N_IDX // C):(cc + 1) * (N_IDX // C)].reshape(-1, 1),
        })
    return in_maps


class _Runner:
    """Cached PJRT executor for the Bass module (axon path, 8 cores)."""

    def __init__(self, nc):
        import jax
        from jax.sharding import Mesh, PartitionSpec, NamedSharding
        from jax.experimental.shard_map import shard_map
        from concourse import bass2jax

        bass2jax.install_neuronx_cc_hook()
        self.jax = jax
        self.nc = nc

        in_names, out_names, out_avals, zero_outs = [], [], [], []
        partition_name = (nc.partition_id_tensor.name
                          if nc.partition_id_tensor else None)
        for alloc in nc.m.functions[0].allocations:
            if not isinstance(alloc, mybir.MemoryLocationSet):
                continue
            name = alloc.memorylocations[0].name
            if alloc.kind == "ExternalInput":
                if name != partition_name:
                    in_names.append(name)
            elif alloc.kind == "ExternalOutput":
                shape = tuple(alloc.tensor_shape)
                dtype = mybir.dt.np(alloc.dtype)
                out_names.append(name)
                out_avals.append(jax.core.ShapedArray(shape, dtype))
                zero_outs.append(np.zeros(shape, dtype))
        self.in_names = list(in_names)
        self.out_names = out_names
        self.out_avals = out_avals
        self.zero_outs = zero_outs
        n_params = len(in_names)
        n_outs = len(out_names)
        all_in_names = list(in_names) + list(out_names)
        if partition_name is not None:
            all_in_names.append(partition_name)
        self._meta = {
            "n_params": n_params,
            "out_avals": out_avals,
            "all_in_names": all_in_names,
            "out_names": out_names,
            "partition_name": partition_name,
        }

        def _body(*args):
            operands = list(args)
            if partition_name is not None:
                operands.append(bass2jax.partition_id_tensor())
            outs = bass2jax._bass_exec_p.bind(
                *operands,
                out_avals=tuple(out_avals),
                in_names=tuple(all_in_names),
                out_names=tuple(out_names),
                lowering_input_output_aliases=(),
                sim_require_finite=True,
                sim_require_nnan=True,
                nc=nc,
            )
            return tuple(outs)

        devices = jax.devices()[:C]
        self.mesh = Mesh(np.asarray(devices), ("core",))
        self.sharding = NamedSharding(self.mesh, PartitionSpec("core"))
        self.sharded = jax.jit(
            shard_map(_body, mesh=self.mesh,
                      in_specs=(PartitionSpec("core"),) * (n_params + n_outs),
                      out_specs=(PartitionSpec("core"),) * n_outs,
                      check_rep=False),
            donate_argnums=tuple(range(n_params, n_params + n_outs)),
            keep_unused=True)
        self.dev_inputs = None

    def put_inputs(self, in_maps):
        """Concat per-core inputs and transfer to devices once."""
        concat = [np.concatenate([np.asarray(m[n]) for m in in_maps], axis=0)
                  for n in self.in_names]
        self.dev_inputs = [self.jax.device_put(a, self.sharding) for a in concat]

    def _zeros(self):
        return [self.jax.device_put(
                    np.zeros((C * z.shape[0], *z.shape[1:]), z.dtype),
                    self.sharding)
                for z in self.zero_outs]

    def run(self):
        outs = self.sharded(*self.dev_inputs, *self._zeros())
        self.jax.block_until_ready(outs)
        return {
            name: np.asarray(outs[i]).reshape(C, *self.out_avals[i].shape)
            for i, name in enumerate(self.out_names)
        }

    def _nodonate(self):
        """Jitted single-exec without donation (safe to call repeatedly)."""
        if getattr(self, "_nodon_fn", None) is not None:
            return self._nodon_fn
        from jax.experimental.shard_map import shard_map
        from jax.sharding import PartitionSpec
        from concourse import bass2jax

        meta = self._meta

        def _body(*args):
            operands = list(args)
            if meta["partition_name"] is not None:
                operands.append(bass2jax.partition_id_tensor())
            return tuple(bass2jax._bass_exec_p.bind(
                *operands,
                out_avals=tuple(meta["out_avals"]),
                in_names=tuple(meta["all_in_names"]),
                out_names=tuple(meta["out_names"]),
                lowering_input_output_aliases=(),
                sim_require_finite=True,
                sim_require_nnan=True,
                nc=self.nc,
            ))

        n_total = meta["n_params"] + len(meta["out_names"])
        self._nodon_fn = self.jax.jit(
            shard_map(_body, mesh=self.mesh,
                      in_specs=(PartitionSpec("core"),) * n_total,
                      out_specs=(PartitionSpec("core"),) * len(meta["out_names"]),
                      check_rep=False),
            keep_unused=True)
        return self._nodon_fn

    def time_floor_diff(self, iters=5):
        """Device time ~= exec wall minus axon dispatch floor (tiny jit)."""
        import time
        zeros = self._zeros()
        tiny = self.jax.device_put(np.ones((8, 8), np.float32),
                                   self.jax.devices()[0])
        ftiny = self.jax.jit(lambda x: x + 1.0)
        self.jax.block_until_ready(ftiny(tiny))
        f = self._nodonate()
        self.jax.block_until_ready(f(*self.dev_inputs, *zeros))

        def best(fn, fargs):
            ts = []
            for _ in range(iters):
                t0 = time.perf_counter()
                self.jax.block_until_ready(fn(*fargs))
                ts.append(time.perf_counter() - t0)
            return min(ts)

        floor = best(ftiny, [tiny])
        t1 = best(f, list(self.dev_inputs) + zeros)
        return max(t1 - floor, 0.0), t1, floor


def _get_runner():
    global _CACHED
    if _CACHED is None:
        nc = _build()
        _CACHED = _Runner(nc)
    return _CACHED


def kernel(A, x0, gcn_w, gcn_b, mlp_w1, mlp_b1, mlp_w2, mlp_b2, nodes_idx):
    runner = _get_runner()
    in_maps = _prep_inputs(A, x0, gcn_w, gcn_b, mlp_w1, mlp_b1, mlp_w2, mlp_b2,
                           nodes_idx)
    runner.put_inputs(in_maps)
    outs = runner.run()
    encode = outs["enc"].reshape(N_IDX, MLP_H)
    out = outs["out"].reshape(N_IDX, N_CLS)
    return encode, out


# revision 14
# speedup vs baseline: 36.1370x; 36.1370x over previous
"""Trainium2 Bass kernel for nn_MultiLevelGCN (3-layer dense GCN + MLP head).

Computation (reference):
    x = x0
    for l in range(3): x = relu((A @ x) @ W_l^T + b_l)
    h = x[nodes_idx]
    encode = relu(h @ w1^T + b1)
    out = softmax(encode @ w2^T + b2)

Sharding: 1-D row partition of A over 8 cores (2048 rows each). Each core
computes its slice of A @ x with the full x; between layers the x slices are
exchanged with chunked AllGathers (4 chunks of 512 rows per layer) so comm
overlaps the next layer's compute. The A operand is passed pre-transposed
and pre-cast to bf16 (A^T column slice, [16384, 2048] per core) so both
matmul operands have the contraction (node) dim on SBUF partitions and the
TensorE runs at 1 cycle/row with the stationary load pipelined (bf16 emits a
separate LDWEIGHTS that the PE queue pulls ahead; the fused 4-byte f32r load
is not pipelined and measures ~1.8x slower). PSUM accumulates fp32. The
small per-layer weight matmul runs f32r so z keeps near-fp32 precision.

Per layer, per core:
    z.T[f, i] = sum_j x[j, f] * A^T[j, i]   (x block stationary, A^T moving)
    xn.T[g, i] = relu(sum_f W^T[f, g] * z.T[f, i] + b[g])
    transpose xn.T -> xn (node-major bf16), store, chunked AllGather.
Head: indirect-DMA row gather of the all-gathered x3 by (permuted)
nodes_idx, MLP + softmax on 1024 rows per core.

DMA: A-stream loads are 512 KB ([512 rows, 512 cols] bf16) alternating
between the two HWDGE rings (sync=qSPDynamicHW, scalar=qActDynamicHW);
x loads are one 1 MB DMA per all-gather chunk. Measured: one ring sustains
~244 GB/s, two rings ~330 GB/s.
"""

import sys

if "/opt/trn_rl_repo" not in sys.path:
    sys.path.insert(0, "/opt/trn_rl_repo")

import ml_dtypes
import numpy as np

import concourse.bass as bass
import concourse.mybir as mybir
import concourse.tile as tile
from concourse import bacc
from concourse.masks import make_identity

N = 16384      # nodes
F = 256        # feature dim
L = 3          # gcn layers
MLP_H = 128    # mlp hidden
N_CLS = 16     # classes
N_IDX = 8192   # labeled nodes
C = 8          # cores
S = N // C     # rows per core = 2048
Q = 4          # all-gather chunks per layer
CH = S // Q    # rows per chunk = 512
P = 128        # partitions
NB = N // P    # j-blocks = 128
IC = 4         # output column chunks per core (512 each)
ICW = S // IC  # 512
NG = N // CH   # 512-row groups of the contraction dim = 32

F32 = mybir.dt.float32
F32R = mybir.dt.float32r
BF16 = mybir.dt.bfloat16
I32 = mybir.dt.int32
NPBF16 = ml_dtypes.bfloat16

AT_BUFS = 8
X_BUFS = 5
TRACE = False
LAST_EXEC_NS = None
LAST_RESULTS = None

_CACHED = None


def _build():
    nc = bacc.Bacc(trn_type="TRN2", target_bir_lowering=False, debug=False,
                   num_devices=C)

    # ---- external I/O (per core) ----
    at_d = nc.dram_tensor("at", [N, S], BF16, kind="ExternalInput")   # A^T slice, bf16
    x0_d = nc.dram_tensor("x0b", [N, F], BF16, kind="ExternalInput")  # x0, bf16
    wt_d = nc.dram_tensor("wt", [L, F, F], F32, kind="ExternalInput")  # W^T per layer [f_in, g_out]
    bias_d = nc.dram_tensor("bias", [L, F, 1], F32, kind="ExternalInput")
    w1t_d = nc.dram_tensor("w1t", [F, MLP_H], BF16, kind="ExternalInput")
    b1_d = nc.dram_tensor("b1", [MLP_H, 1], F32, kind="ExternalInput")
    w2t_d = nc.dram_tensor("w2t", [MLP_H, N_CLS], F32, kind="ExternalInput")
    b2_d = nc.dram_tensor("b2", [1, N_CLS], F32, kind="ExternalInput")
    idx_d = nc.dram_tensor("idxp", [N_IDX // C, 1], I32, kind="ExternalInput")
    enc_d = nc.dram_tensor("enc", [N_IDX // C, MLP_H], F32, kind="ExternalOutput")
    out_d = nc.dram_tensor("out", [N_IDX // C, N_CLS], F32, kind="ExternalOutput")

    # ---- internal DRAM (bf16 activations) ----
    xsl = [[nc.dram_tensor(f"xsl_{l}_{q}", [CH, F], BF16) for q in range(Q)]
           for l in range(L)]
    xg = {l: [nc.dram_tensor(f"xg_{l}_{q}", [C * CH, F], BF16, addr_space="Shared")
              for q in range(Q)]
          for l in (1, 2)}
    # Final gathered x3: one tensor so indirect DMA gathers from offset 0.
    # Row layout: q*4096 + c*512 + r  for global node j = c*2048 + q*512 + r.
    xg3_d = nc.dram_tensor("xg3", [N, F], BF16, addr_space="Shared")

    rg = [list(range(C))]

    dma_engines = [nc.sync, nc.scalar]  # the two HWDGE rings

    with tile.TileContext(nc) as tc:
        with (
            tc.tile_pool(name="xres", bufs=X_BUFS) as x_pool,
            tc.tile_pool(name="at", bufs=AT_BUFS) as at_pool,
            tc.tile_pool(name="zt", bufs=4) as zt_pool,
            tc.tile_pool(name="xnt", bufs=4) as xnt_pool,
            tc.tile_pool(name="xn", bufs=4) as xn_pool,
            tc.tile_pool(name="wconst", bufs=1) as w_pool,
            tc.tile_pool(name="head", bufs=4) as h_pool,
            tc.tile_pool(name="headc", bufs=1) as hc_pool,
            tc.tile_pool(name="sm", bufs=8) as sm_pool,
            tc.tile_pool(name="pz", bufs=4, space="PSUM") as pz_pool,
            tc.tile_pool(name="px", bufs=2, space="PSUM") as px_pool,
            tc.tile_pool(name="pt", bufs=2, space="PSUM") as pt_pool,
        ):
            # ---- constants ----
            ident_b = w_pool.tile([P, P], BF16, tag="identb")
            make_identity(nc, ident_b[:])
            ident_f = w_pool.tile([P, P], F32, tag="identf")
            make_identity(nc, ident_f[:])
            wt_sb = w_pool.tile([P, L * 2, F], F32R, tag="wt")
            for l in range(L):
                for fb in range(2):
                    nc.sync.dma_start(
                        out=wt_sb[:, l * 2 + fb, :],
                        in_=wt_d[l, fb * P:(fb + 1) * P, :].bitcast(F32R))
            bias_sb = w_pool.tile([P, L * 2], F32, tag="bias")
            for l in range(L):
                for gb in range(2):
                    nc.sync.dma_start(
                        out=bias_sb[:, l * 2 + gb:l * 2 + gb + 1],
                        in_=bias_d[l, gb * P:(gb + 1) * P, :])
            w1t_sb = w_pool.tile([P, 2, MLP_H], BF16, tag="w1t")
            for fb in range(2):
                nc.sync.dma_start(out=w1t_sb[:, fb, :],
                                  in_=w1t_d[fb * P:(fb + 1) * P, :])
            b1_sb = w_pool.tile([P, 1], F32, tag="b1")
            nc.sync.dma_start(out=b1_sb[:, :], in_=b1_d[:, :])
            w2t_sb = w_pool.tile([P, N_CLS], F32, tag="w2t")
            nc.sync.dma_start(out=w2t_sb[:, :], in_=w2t_d[:, :])
            b2_sb = w_pool.tile([1, N_CLS], F32, tag="b2")
            nc.sync.dma_start(out=b2_sb[:, :], in_=b2_d[:, :])
            ones_sb = w_pool.tile([1, P], F32, tag="ones")
            nc.gpsimd.memset(ones_sb[:, :], 1.0)

            # ---- GCN layers ----
            # Contraction rows are processed in 512-row groups. Group g covers
            # global nodes [g*512, (g+1)*512). For l>=1 availability order is
            # chunk-q major (gathered chunk q lands first); layer 0 is natural.
            for l in range(L):
                if l == 0:
                    g_list = list(range(NG))
                else:
                    g_list = [c * Q + q for q in range(Q) for c in range(C)]

                # x resident tiles: one [128, 32, 256] bf16 tile per source
                # region (layer 0: x0 quarters; l>=1: gathered chunk q).
                xt = {}
                for i, q in enumerate(range(Q)):
                    t = x_pool.tile([P, (N // Q) // P, F], BF16, tag="x", name="x")
                    if l == 0:
                        src = x0_d[q * (N // Q):(q + 1) * (N // Q), :]
                    else:
                        src = xg[l][q][:, :]
                    # DRAM rows (s*128 + p, f) -> SBUF (p, s, f)
                    dma_engines[i % 2].dma_start(
                        out=t[:], in_=src.rearrange("(s p) f -> p s f", p=P))
                    xt[q] = t

                def x_slot(j0):
                    # (tile, slot) holding global nodes [j0, j0+128)
                    if l == 0:
                        return xt[j0 // (N // Q)], (j0 % (N // Q)) // P
                    c, rem = divmod(j0, S)
                    q, r = divmod(rem, CH)
                    return xt[q], c * (CH // P) + r // P

                for ic in range(IC):
                    pz = [pz_pool.tile([P, ICW], F32, tag="pz", name="pz")
                          for _ in range(2)]
                    for gi, g in enumerate(g_list):
                        at_t = at_pool.tile([P, CH // P, ICW], BF16, tag="at",
                                            name="at")
                        dma_engines[gi % 2].dma_start(
                            out=at_t[:],
                            in_=at_d[g * CH:(g + 1) * CH,
                                     ic * ICW:(ic + 1) * ICW]
                            .rearrange("(r p) c -> p r c", p=P))
                        for rb in range(CH // P):
                            xtile, slot = x_slot(g * CH + rb * P)
                            for fb in range(2):
                                nc.tensor.matmul(
                                    out=pz[fb][:],
                                    lhsT=xtile[:, slot, fb * P:(fb + 1) * P],
                                    rhs=at_t[:, rb, :],
                                    start=(gi == 0 and rb == 0),
                                    stop=(gi == NG - 1 and rb == CH // P - 1))

                    zt = []
                    for fb in range(2):
                        z = zt_pool.tile([P, ICW], F32R, tag="zt")
                        nc.vector.tensor_copy(out=z[:], in_=pz[fb][:])
                        zt.append(z)

                    xnt = []
                    for gb in range(2):
                        px = px_pool.tile([P, ICW], F32, tag="px")
                        for fb in range(2):
                            nc.tensor.matmul(
                                out=px[:],
                                lhsT=wt_sb[:, l * 2 + fb, gb * P:(gb + 1) * P],
                                rhs=zt[fb][:],
                                start=(fb == 0), stop=(fb == 1))
                        xt_out = xnt_pool.tile([P, ICW], BF16, tag="xnt")
                        nc.scalar.activation(
                            out=xt_out[:], in_=px[:],
                            func=mybir.ActivationFunctionType.Relu,
                            bias=bias_sb[:, l * 2 + gb:l * 2 + gb + 1])
                        xnt.append(xt_out)

                    for isub in range(ICW // P):
                        xn = xn_pool.tile([P, F], BF16, tag="xn")
                        for gb in range(2):
                            ptt = pt_pool.tile([P, P], BF16, tag="pt")
                            nc.tensor.transpose(
                                out=ptt[:], in_=xnt[gb][:, isub * P:(isub + 1) * P],
                                identity=ident_b[:])
                            nc.vector.tensor_copy(
                                out=xn[:, gb * P:(gb + 1) * P], in_=ptt[:])
                        nc.sync.dma_start(
                            out=xsl[l][ic][isub * P:(isub + 1) * P, :], in_=xn[:])

                    outs = (xg[l + 1][ic][:, :] if l < L - 1
                            else xg3_d[ic * C * CH:(ic + 1) * C * CH, :])
                    nc.gpsimd.collective_compute(
                        "AllGather", mybir.AluOpType.bypass,
                        replica_groups=rg,
                        ins=[xsl[l][ic][:, :]],
                        outs=[outs])

            # ---- head: gather + MLP + softmax ----
            n_ch = (N_IDX // C) // P  # 8 chunks of 128 indices
            ht = [hc_pool.tile([P, n_ch * P], BF16, tag=f"ht{fb}", name=f"ht{fb}")
                  for fb in range(2)]
            for ch in range(n_ch):
                idx_t = h_pool.tile([P, 1], I32, tag="idx")
                nc.sync.dma_start(out=idx_t[:], in_=idx_d[ch * P:(ch + 1) * P, :])
                h = h_pool.tile([P, F], BF16, tag="h")
                nc.gpsimd.indirect_dma_start(
                    out=h[:], out_offset=None,
                    in_=xg3_d[:, :],
                    in_offset=bass.IndirectOffsetOnAxis(ap=idx_t[:, :1], axis=0))
                for fb in range(2):
                    ptt = pt_pool.tile([P, P], BF16, tag="pt")
                    nc.tensor.transpose(out=ptt[:], in_=h[:, fb * P:(fb + 1) * P],
                                        identity=ident_b[:])
                    nc.vector.tensor_copy(out=ht[fb][:, ch * P:(ch + 1) * P],
                                          in_=ptt[:])

            et = hc_pool.tile([P, n_ch * P], F32, tag="et")  # encode.T [m, i2]
            for i2c in range(2):
                pe = px_pool.tile([P, ICW], F32, tag="px")
                for fb in range(2):
                    nc.tensor.matmul(
                        out=pe[:],
                        lhsT=w1t_sb[:, fb, :],
                        rhs=ht[fb][:, i2c * ICW:(i2c + 1) * ICW],
                        start=(fb == 0), stop=(fb == 1))
                nc.scalar.activation(
                    out=et[:, i2c * ICW:(i2c + 1) * ICW], in_=pe[:],
                    func=mybir.ActivationFunctionType.Relu,
                    bias=b1_sb[:, 0:1])

            for ch in range(n_ch):
                ptt = pt_pool.tile([P, P], F32, tag="pt", name="ptf")
                nc.tensor.transpose(out=ptt[:], in_=et[:, ch * P:(ch + 1) * P],
                                    identity=ident_f[:])
                enc_t = h_pool.tile([P, MLP_H], F32, tag="enc")
                nc.vector.tensor_copy(out=enc_t[:], in_=ptt[:])
                nc.sync.dma_start(out=enc_d[ch * P:(ch + 1) * P, :], in_=enc_t[:])

                pl = pt_pool.tile([P, N_CLS], F32, tag="pt", name="ptf")
                nc.tensor.matmul(out=pl[:], lhsT=ones_sb[:, :], rhs=b2_sb[:, :],
                                 start=True, stop=False, skip_group_check=True)
                nc.tensor.matmul(out=pl[:], lhsT=et[:, ch * P:(ch + 1) * P],
                                 rhs=w2t_sb[:, :],
                                 start=False, stop=True, skip_group_check=True)
                nmax = sm_pool.tile([P, 1], F32, tag="nmax")
                nc.vector.tensor_reduce(out=nmax[:], in_=pl[:],
                                        axis=mybir.AxisListType.X,
                                        op=mybir.AluOpType.max, negate=True)
                ex = sm_pool.tile([P, N_CLS], F32, tag="ex")
                ssum = sm_pool.tile([P, 1], F32, tag="ssum")
                nc.scalar.activation(out=ex[:], in_=pl[:],
                                     func=mybir.ActivationFunctionType.Exp,
                                     bias=nmax[:, 0:1],
                                     accum_out=ssum[:, 0:1])
                rs = sm_pool.tile([P, 1], F32, tag="rs")
                nc.vector.reciprocal(out=rs[:], in_=ssum[:])
                ot = sm_pool.tile([P, N_CLS], F32, tag="ot")
                nc.scalar.activation(out=ot[:], in_=ex[:],
                                     func=mybir.ActivationFunctionType.Copy,
                                     scale=rs[:, 0:1])
                nc.sync.dma_start(out=out_d[ch * P:(ch + 1) * P, :], in_=ot[:])

    nc.compile()
    return nc


def _prep_inputs(A, x0, gcn_w, gcn_b, mlp_w1, mlp_b1, mlp_w2, mlp_b2, nodes_idx):
    A = np.asarray(A, dtype=np.float32)
    Ab = A.astype(NPBF16)
    x0b = np.ascontiguousarray(np.asarray(x0, dtype=np.float32)).astype(NPBF16)
    wt = np.ascontiguousarray(np.asarray(gcn_w, np.float32).transpose(0, 2, 1))
    bias = np.ascontiguousarray(np.asarray(gcn_b, np.float32).reshape(L, F, 1))
    w1t = np.ascontiguousarray(np.asarray(mlp_w1, np.float32).T).astype(NPBF16)
    b1 = np.ascontiguousarray(np.asarray(mlp_b1, np.float32).reshape(MLP_H, 1))
    w2t = np.ascontiguousarray(np.asarray(mlp_w2, np.float32).T)
    b2 = np.ascontiguousarray(np.asarray(mlp_b2, np.float32).reshape(1, N_CLS))
    idx = np.asarray(nodes_idx).astype(np.int64)
    # permute indices into the xg3 row layout: q*4096 + c*512 + r
    c = idx // S
    q = (idx % S) // CH
    r = idx % CH
    idxp = (q * (C * CH) + c * CH + r).astype(np.int32)

    in_maps = []
    for cc in range(C):
        at_c = np.ascontiguousarray(Ab[cc * S:(cc + 1) * S, :].T)
        in_maps.append({
            "at": at_c,
            "x0b": x0b,
            "wt": wt,
            "bias": bias,
            "w1t": w1t,
            "b1": b1,
            "w2t": w2t,
            "b2": b2,
            "idxp": idxp[cc * (N_IDX // C):(cc + 1) * (N_IDX // C)].reshape(-1, 1),
        })
    return in_maps


class _Runner:
    """Cached PJRT executor for the Bass module (axon path, 8 cores)."""

    def __init__(self, nc):
        import jax
        from jax.sharding import Mesh, PartitionSpec, NamedSharding
        from jax.experimental.shard_map import shard_map
        from concourse import bass2jax

        bass2jax.install_neuronx_cc_hook()
        self.jax = jax
        self.nc = nc

        in_names, out_names, out_avals, zero_outs = [], [], [], []
        partition_name = (nc.partition_id_tensor.name
                          if nc.partition_id_tensor else None)
        for alloc in nc.m.functions[0].allocations:
            if not isinstance(alloc, mybir.MemoryLocationSet):
                continue
            name = alloc.memorylocations[0].name
            if alloc.kind == "ExternalInput":
                if name != partition_name:
                    in_names.append(name)
            elif alloc.kind == "ExternalOutput":
                shape = tuple(alloc.tensor_shape)
                dtype = mybir.dt.np(alloc.dtype)
                out_names.append(name)
                out_avals.append(jax.core.ShapedArray(shape, dtype))
                zero_outs.append(np.zeros(shape, dtype))
        self.in_names = list(in_names)
        self.out_names = out_names
        self.out_avals = out_avals
        self.zero_outs = zero_outs
        n_params = len(in_names)
        n_outs = len(out_names)
        all_in_names = list(in_names) + list(out_names)
        if partition_name is not None:
            all_in_names.append(partition_name)
        self._meta = {
            "n_params": n_params,
            "out_avals": out_avals,
            "all_in_names": all_in_names,
            "out_names": out_names,
            "partition_name": partition_name,
        }

        def _body(*args):
            operands = list(args)
            if partition_name is not None:
                operands.append(bass2jax.partition_id_tensor())
            outs = bass2jax._bass_exec_p.bind(
                *operands,
                out_avals=tuple(out_avals),
                in_names=tuple(all_in_names),
                out_names=tuple(out_names),
                lowering_input_output_aliases=(),
                sim_require_finite=True,
                sim_require_nnan=True,
                nc=nc,
            )
            return tuple(outs)

        devices = jax.devices()[:C]
        self.mesh = Mesh(np.asarray(devices), ("core",))
        self.sharding = NamedSharding(self.mesh, PartitionSpec("core"))
        self.sharded = jax.jit(
            shard_map(_body, mesh=self.mesh,
                      in_specs=(PartitionSpec("core"),) * (n_params + n_outs),
                      out_specs=(PartitionSpec("core"),) * n_outs,
                      check_rep=False),
            donate_argnums=tuple(range(n_params, n_params + n_outs)),
            keep_unused=True)
        self.dev_inputs = None

    def put_inputs(self, in_maps):
        """Concat per-core inputs and transfer to devices once."""
        concat = [np.concatenate([np.asarray(m[n]) for m in in_maps], axis=0)
                  for n in self.in_names]
        self.dev_inputs = [self.jax.device_put(a, self.sharding) for a in concat]

    def _zeros(self):
        return [self.jax.device_put(
                    np.zeros((C * z.shape[0], *z.shape[1:]), z.dtype),
                    self.sharding)
                for z in self.zero_outs]

    def run(self):
        outs = self.sharded(*self.dev_inputs, *self._zeros())
        self.jax.block_until_ready(outs)
        return {
            name: np.asarray(outs[i]).reshape(C, *self.out_avals[i].shape)
            for i, name in enumerate(self.out_names)
        }

    def _nodonate(self):
        """Jitted single-exec without donation (safe to call repeatedly)."""
        if getattr(self, "_nodon_fn", None) is not None:
            return self._nodon_fn
        from jax.experimental.shard_map import shard_map
        from jax.sharding import PartitionSpec
        from concourse import bass2jax

        meta = self._meta

        def _body(*args):
            operands = list(args)
            if meta["partition_name"] is not None:
                operands.append(bass2jax.partition_id_tensor())
            return tuple(bass2jax._bass_exec_p.bind(
                *operands,
                out_avals=tuple(meta["out_avals"]),
                in_names=tuple(meta["all_in_names"]),
                out_names=tuple(meta["out_names"]),
                lowering_input_output_aliases=(),
                sim_require_finite=True,
                sim_require_nnan=True,
                nc=self.nc,
            ))

        n_total = meta["n_params"] + len(meta["out_names"])
        self._nodon_fn = self.jax.jit(
            shard_map(_body, mesh=self.mesh,
                      in_specs=(PartitionSpec("core"),) * n_total,
                      out_specs=(PartitionSpec("core"),) * len(meta["out_names"]),
                      check_rep=False),
            keep_unused=True)
        return self._nodon_fn

    def time_floor_diff(self, iters=5):
        """Device time ~= exec wall minus axon dispatch floor (tiny jit)."""
        import time
        zeros = self._zeros()
        tiny = self.jax.device_put(np.ones((8, 8), np.float32),
                                   self.jax.devices()[0])
        ftiny = self.jax.jit(lambda x: x + 1.0)
        self.jax.block_until_ready(ftiny(tiny))
        f = self._nodonate()
        self.jax.block_until_ready(f(*self.dev_inputs, *zeros))

        def best(fn, fargs):
            ts = []
            for _ in range(iters):
                t0 = time.perf_counter()
                self.jax.block_until_ready(fn(*fargs))
                ts.append(time.perf_counter() - t0)
            return min(ts)

        floor = best(ftiny, [tiny])
        t1 = best(f, list(self.dev_inputs) + zeros)
        return max(t1 - floor, 0.0), t1, floor

    def time_pipelined(self, k=8, iters=5):
        """Dispatch k execs without blocking, block once: if dispatch is
        async, slope over k removes the per-call round-trip latency."""
        import time
        zeros = self._zeros()
        f = self._nodonate()
        args = list(self.dev_inputs) + zeros
        self.jax.block_until_ready(f(*args))

        def run_k(kk):
            ts = []
            for _ in range(iters):
                t0 = time.perf_counter()
                outs = None
                for _ in range(kk):
                    outs = f(*args)
                self.jax.block_until_ready(outs)
                ts.append(time.perf_counter() - t0)
            return min(ts)

        t1 = run_k(1)
        tk = run_k(k)
        return (tk - t1) / (k - 1), t1, tk


def _get_runner():
    global _CACHED
    if _CACHED is None:
        nc = _build()
        _CACHED = _Runner(nc)
    return _CACHED


def kernel(A, x0, gcn_w, gcn_b, mlp_w1, mlp_b1, mlp_w2, mlp_b2, nodes_idx):
    runner = _get_runner()
    in_maps = _prep_inputs(A, x0, gcn_w, gcn_b, mlp_w1, mlp_b1, mlp_w2, mlp_b2,
                           nodes_idx)
    runner.put_inputs(in_maps)
    outs = runner.run()
    encode = outs["enc"].reshape(N_IDX, MLP_H)
    out = outs["out"].reshape(N_IDX, N_CLS)
    return encode, out


# revision 16
# speedup vs baseline: 39.5888x; 1.0955x over previous
"""Trainium2 Bass kernel for nn_MultiLevelGCN (3-layer dense GCN + MLP head).

Computation (reference):
    x = x0
    for l in range(3): x = relu((A @ x) @ W_l^T + b_l)
    h = x[nodes_idx]
    encode = relu(h @ w1^T + b1)
    out = softmax(encode @ w2^T + b2)

Sharding: 1-D row partition of A over 8 cores (2048 rows each). Each core
computes its slice of A @ x with the full x; between layers the x slices are
exchanged with chunked AllGathers (4 chunks of 512 rows per layer) so comm
overlaps the next layer's compute. The A operand is passed pre-transposed
and pre-cast to bf16 (A^T column slice, [16384, 2048] per core) so both
matmul operands have the contraction (node) dim on SBUF partitions and the
TensorE runs at 1 cycle/row with the stationary load pipelined (bf16 emits a
separate LDWEIGHTS that the PE queue pulls ahead; the fused 4-byte f32r load
is not pipelined and measures ~1.8x slower). PSUM accumulates fp32. The
small per-layer weight matmul runs f32r so z keeps near-fp32 precision.

Per layer, per core:
    z.T[f, i] = sum_j x[j, f] * A^T[j, i]   (x block stationary, A^T moving)
    xn.T[g, i] = relu(sum_f W^T[f, g] * z.T[f, i] + b[g])
    transpose xn.T -> xn (node-major bf16), store, chunked AllGather.
Head: indirect-DMA row gather of the all-gathered x3 by (permuted)
nodes_idx, MLP + softmax on 1024 rows per core.

DMA: A-stream loads are 512 KB ([512 rows, 512 cols] bf16) alternating
between the two HWDGE rings (sync=qSPDynamicHW, scalar=qActDynamicHW);
x loads are one 1 MB DMA per all-gather chunk. Measured: one ring sustains
~244 GB/s, two rings ~330 GB/s.
"""

import sys

if "/opt/trn_rl_repo" not in sys.path:
    sys.path.insert(0, "/opt/trn_rl_repo")

import ml_dtypes
import numpy as np

import concourse.bass as bass
import concourse.mybir as mybir
import concourse.tile as tile
from concourse import bacc
from concourse.masks import make_identity

N = 16384      # nodes
F = 256        # feature dim
L = 3          # gcn layers
MLP_H = 128    # mlp hidden
N_CLS = 16     # classes
N_IDX = 8192   # labeled nodes
C = 8          # cores
S = N // C     # rows per core = 2048
Q = 4          # all-gather chunks per layer
CH = S // Q    # rows per chunk = 512
P = 128        # partitions
NB = N // P    # j-blocks = 128
IC = 4         # output column chunks per core (512 each)
ICW = S // IC  # 512
NG = N // CH   # 512-row groups of the contraction dim = 32

F32 = mybir.dt.float32
F32R = mybir.dt.float32r
BF16 = mybir.dt.bfloat16
I32 = mybir.dt.int32
NPBF16 = ml_dtypes.bfloat16

AT_BUFS = 6
X_BUFS = 5
TRACE = False
LAST_EXEC_NS = None
LAST_RESULTS = None

_CACHED = None


def _build():
    nc = bacc.Bacc(trn_type="TRN2", target_bir_lowering=False, debug=False,
                   num_devices=C)

    # ---- external I/O (per core) ----
    at_d = nc.dram_tensor("at", [N, S], BF16, kind="ExternalInput")   # A^T slice, bf16
    x0_d = nc.dram_tensor("x0b", [N, F], BF16, kind="ExternalInput")  # x0, bf16
    wt_d = nc.dram_tensor("wt", [L, F, F], F32, kind="ExternalInput")  # W^T per layer [f_in, g_out]
    bias_d = nc.dram_tensor("bias", [L, F, 1], F32, kind="ExternalInput")
    w1t_d = nc.dram_tensor("w1t", [F, MLP_H], BF16, kind="ExternalInput")
    b1_d = nc.dram_tensor("b1", [MLP_H, 1], F32, kind="ExternalInput")
    w2t_d = nc.dram_tensor("w2t", [MLP_H, N_CLS], F32, kind="ExternalInput")
    b2_d = nc.dram_tensor("b2", [1, N_CLS], F32, kind="ExternalInput")
    idx_d = nc.dram_tensor("idxp", [N_IDX // C, 1], I32, kind="ExternalInput")
    enc_d = nc.dram_tensor("enc", [N_IDX // C, MLP_H], F32, kind="ExternalOutput")
    out_d = nc.dram_tensor("out", [N_IDX // C, N_CLS], F32, kind="ExternalOutput")

    # ---- internal DRAM (bf16 activations) ----
    xsl = [[nc.dram_tensor(f"xsl_{l}_{q}", [CH, F], BF16) for q in range(Q)]
           for l in range(L)]
    xg = {l: [nc.dram_tensor(f"xg_{l}_{q}", [C * CH, F], BF16, addr_space="Shared")
              for q in range(Q)]
          for l in (1, 2)}
    # Final gathered x3: one tensor so indirect DMA gathers from offset 0.
    # Row layout: q*4096 + c*512 + r  for global node j = c*2048 + q*512 + r.
    xg3_d = nc.dram_tensor("xg3", [N, F], BF16, addr_space="Shared")

    rg = [list(range(C))]

    dma_engines = [nc.sync, nc.scalar]  # the two HWDGE rings

    with tile.TileContext(nc) as tc:
        with (
            tc.tile_pool(name="xres", bufs=X_BUFS) as x_pool,
            tc.tile_pool(name="at", bufs=AT_BUFS) as at_pool,
            tc.tile_pool(name="zt", bufs=4) as zt_pool,
            tc.tile_pool(name="xnt", bufs=4) as xnt_pool,
            tc.tile_pool(name="xn", bufs=4) as xn_pool,
            tc.tile_pool(name="wconst", bufs=1) as w_pool,
            tc.tile_pool(name="head", bufs=4) as h_pool,
            tc.tile_pool(name="headc", bufs=1) as hc_pool,
            tc.tile_pool(name="sm", bufs=8) as sm_pool,
            tc.tile_pool(name="pz", bufs=4, space="PSUM") as pz_pool,
            tc.tile_pool(name="px", bufs=2, space="PSUM") as px_pool,
            tc.tile_pool(name="pt", bufs=2, space="PSUM") as pt_pool,
        ):
            # ---- constants ----
            ident_b = w_pool.tile([P, P], BF16, tag="identb")
            make_identity(nc, ident_b[:])
            ident_f = w_pool.tile([P, P], F32, tag="identf")
            make_identity(nc, ident_f[:])
            wt_sb = w_pool.tile([P, L * 2, F], F32R, tag="wt")
            for l in range(L):
                for fb in range(2):
                    nc.sync.dma_start(
                        out=wt_sb[:, l * 2 + fb, :],
                        in_=wt_d[l, fb * P:(fb + 1) * P, :].bitcast(F32R))
            bias_sb = w_pool.tile([P, L * 2], F32, tag="bias")
            for l in range(L):
                for gb in range(2):
                    nc.sync.dma_start(
                        out=bias_sb[:, l * 2 + gb:l * 2 + gb + 1],
                        in_=bias_d[l, gb * P:(gb + 1) * P, :])
            w1t_sb = w_pool.tile([P, 2, MLP_H], BF16, tag="w1t")
            for fb in range(2):
                nc.sync.dma_start(out=w1t_sb[:, fb, :],
                                  in_=w1t_d[fb * P:(fb + 1) * P, :])
            b1_sb = w_pool.tile([P, 1], F32, tag="b1")
            nc.sync.dma_start(out=b1_sb[:, :], in_=b1_d[:, :])
            w2t_sb = w_pool.tile([P, N_CLS], F32, tag="w2t")
            nc.sync.dma_start(out=w2t_sb[:, :], in_=w2t_d[:, :])
            b2_sb = w_pool.tile([1, N_CLS], F32, tag="b2")
            nc.sync.dma_start(out=b2_sb[:, :], in_=b2_d[:, :])
            ones_sb = w_pool.tile([1, P], F32, tag="ones")
            nc.gpsimd.memset(ones_sb[:, :], 1.0)

            # ---- GCN layers ----
            # Contraction rows are processed in 512-row groups. Group g covers
            # global nodes [g*512, (g+1)*512). For l>=1 availability order is
            # chunk-q major (gathered chunk q lands first); layer 0 is natural.
            for l in range(L):
                if l == 0:
                    g_list = list(range(NG))
                else:
                    g_list = [c * Q + q for q in range(Q) for c in range(C)]

                # x resident tiles: one [128, 32, 256] bf16 tile per source
                # region (layer 0: x0 quarters; l>=1: gathered chunk q).
                xt = {}
                for i, q in enumerate(range(Q)):
                    t = x_pool.tile([P, (N // Q) // P, F], BF16, tag="x", name="x")
                    if l == 0:
                        src = x0_d[q * (N // Q):(q + 1) * (N // Q), :]
                    else:
                        src = xg[l][q][:, :]
                    # DRAM rows (s*128 + p, f) -> SBUF (p, s, f)
                    dma_engines[i % 2].dma_start(
                        out=t[:], in_=src.rearrange("(s p) f -> p s f", p=P))
                    xt[q] = t

                def x_slot(j0):
                    # (tile, slot) holding global nodes [j0, j0+128)
                    if l == 0:
                        return xt[j0 // (N // Q)], (j0 % (N // Q)) // P
                    c, rem = divmod(j0, S)
                    q, r = divmod(rem, CH)
                    return xt[q], c * (CH // P) + r // P

                # Two output super-chunks of 1024 cols; each A^T tile is 1 MB
                # ([512 rows, 1024 cols] bf16, 2 KB segments) consumed by four
                # PSUM accumulation groups (2 sub-chunks x 2 feature blocks).
                for icp in range(IC // 2):
                    pz = [[pz_pool.tile([P, ICW], F32, tag="pz", name="pz")
                           for _ in range(2)] for _ in range(2)]  # [sub][fb]
                    for gi, g in enumerate(g_list):
                        at_t = at_pool.tile([P, CH // P, 2 * ICW], BF16,
                                            tag="at", name="at")
                        dma_engines[gi % 2].dma_start(
                            out=at_t[:],
                            in_=at_d[g * CH:(g + 1) * CH,
                                     icp * 2 * ICW:(icp + 1) * 2 * ICW]
                            .rearrange("(r p) c -> p r c", p=P))
                        for rb in range(CH // P):
                            xtile, slot = x_slot(g * CH + rb * P)
                            for sub in range(2):
                                for fb in range(2):
                                    nc.tensor.matmul(
                                        out=pz[sub][fb][:],
                                        lhsT=xtile[:, slot, fb * P:(fb + 1) * P],
                                        rhs=at_t[:, rb, sub * ICW:(sub + 1) * ICW],
                                        start=(gi == 0 and rb == 0),
                                        stop=(gi == NG - 1 and rb == CH // P - 1))

                    for sub in range(2):
                        ic = icp * 2 + sub
                        zt = []
                        for fb in range(2):
                            z = zt_pool.tile([P, ICW], F32R, tag="zt")
                            nc.vector.tensor_copy(out=z[:], in_=pz[sub][fb][:])
                            zt.append(z)

                        xnt = []
                        for gb in range(2):
                            px = px_pool.tile([P, ICW], F32, tag="px")
                            for fb in range(2):
                                nc.tensor.matmul(
                                    out=px[:],
                                    lhsT=wt_sb[:, l * 2 + fb, gb * P:(gb + 1) * P],
                                    rhs=zt[fb][:],
                                    start=(fb == 0), stop=(fb == 1))
                            xt_out = xnt_pool.tile([P, ICW], BF16, tag="xnt")
                            nc.scalar.activation(
                                out=xt_out[:], in_=px[:],
                                func=mybir.ActivationFunctionType.Relu,
                                bias=bias_sb[:, l * 2 + gb:l * 2 + gb + 1])
                            xnt.append(xt_out)

                        for isub in range(ICW // P):
                            xn = xn_pool.tile([P, F], BF16, tag="xn")
                            for gb in range(2):
                                ptt = pt_pool.tile([P, P], BF16, tag="pt")
                                nc.tensor.transpose(
                                    out=ptt[:],
                                    in_=xnt[gb][:, isub * P:(isub + 1) * P],
                                    identity=ident_b[:])
                                nc.vector.tensor_copy(
                                    out=xn[:, gb * P:(gb + 1) * P], in_=ptt[:])
                            nc.gpsimd.dma_start(
                                out=xsl[l][ic][isub * P:(isub + 1) * P, :],
                                in_=xn[:])

                        outs = (xg[l + 1][ic][:, :] if l < L - 1
                                else xg3_d[ic * C * CH:(ic + 1) * C * CH, :])
                        nc.gpsimd.collective_compute(
                            "AllGather", mybir.AluOpType.bypass,
                            replica_groups=rg,
                            ins=[xsl[l][ic][:, :]],
                            outs=[outs])

            # ---- head: gather + MLP + softmax ----
            n_ch = (N_IDX // C) // P  # 8 chunks of 128 indices
            ht = [hc_pool.tile([P, n_ch * P], BF16, tag=f"ht{fb}", name=f"ht{fb}")
                  for fb in range(2)]
            for ch in range(n_ch):
                idx_t = h_pool.tile([P, 1], I32, tag="idx")
                nc.sync.dma_start(out=idx_t[:], in_=idx_d[ch * P:(ch + 1) * P, :])
                h = h_pool.tile([P, F], BF16, tag="h")
                nc.gpsimd.indirect_dma_start(
                    out=h[:], out_offset=None,
                    in_=xg3_d[:, :],
                    in_offset=bass.IndirectOffsetOnAxis(ap=idx_t[:, :1], axis=0))
                for fb in range(2):
                    ptt = pt_pool.tile([P, P], BF16, tag="pt")
                    nc.tensor.transpose(out=ptt[:], in_=h[:, fb * P:(fb + 1) * P],
                                        identity=ident_b[:])
                    nc.vector.tensor_copy(out=ht[fb][:, ch * P:(ch + 1) * P],
                                          in_=ptt[:])

            et = hc_pool.tile([P, n_ch * P], F32, tag="et")  # encode.T [m, i2]
            for i2c in range(2):
                pe = px_pool.tile([P, ICW], F32, tag="px")
                for fb in range(2):
                    nc.tensor.matmul(
                        out=pe[:],
                        lhsT=w1t_sb[:, fb, :],
                        rhs=ht[fb][:, i2c * ICW:(i2c + 1) * ICW],
                        start=(fb == 0), stop=(fb == 1))
                nc.scalar.activation(
                    out=et[:, i2c * ICW:(i2c + 1) * ICW], in_=pe[:],
                    func=mybir.ActivationFunctionType.Relu,
                    bias=b1_sb[:, 0:1])

            for ch in range(n_ch):
                ptt = pt_pool.tile([P, P], F32, tag="pt", name="ptf")
                nc.tensor.transpose(out=ptt[:], in_=et[:, ch * P:(ch + 1) * P],
                                    identity=ident_f[:])
                enc_t = h_pool.tile([P, MLP_H], F32, tag="enc")
                nc.vector.tensor_copy(out=enc_t[:], in_=ptt[:])
                nc.sync.dma_start(out=enc_d[ch * P:(ch + 1) * P, :], in_=enc_t[:])

                pl = pt_pool.tile([P, N_CLS], F32, tag="pt", name="ptf")
                nc.tensor.matmul(out=pl[:], lhsT=ones_sb[:, :], rhs=b2_sb[:, :],
                                 start=True, stop=False, skip_group_check=True)
                nc.tensor.matmul(out=pl[:], lhsT=et[:, ch * P:(ch + 1) * P],
                                 rhs=w2t_sb[:, :],
                                 start=False, stop=True, skip_group_check=True)
                nmax = sm_pool.tile([P, 1], F32, tag="nmax")
                nc.vector.tensor_reduce(out=nmax[:], in_=pl[:],
                                        axis=mybir.AxisListType.X,
                                        op=mybir.AluOpType.max, negate=True)
                ex = sm_pool.tile([P, N_CLS], F32, tag="ex")
                ssum = sm_pool.tile([P, 1], F32, tag="ssum")
                nc.scalar.activation(out=ex[:], in_=pl[:],
                                     func=mybir.ActivationFunctionType.Exp,
                                     bias=nmax[:, 0:1],
                                     accum_out=ssum[:, 0:1])
                rs = sm_pool.tile([P, 1], F32, tag="rs")
                nc.vector.reciprocal(out=rs[:], in_=ssum[:])
                ot = sm_pool.tile([P, N_CLS], F32, tag="ot")
                nc.scalar.activation(out=ot[:], in_=ex[:],
                                     func=mybir.ActivationFunctionType.Copy,
                                     scale=rs[:, 0:1])
                nc.sync.dma_start(out=out_d[ch * P:(ch + 1) * P, :], in_=ot[:])

    nc.compile()
    return nc


def _prep_inputs(A, x0, gcn_w, gcn_b, mlp_w1, mlp_b1, mlp_w2, mlp_b2, nodes_idx):
    A = np.asarray(A, dtype=np.float32)
    Ab = A.astype(NPBF16)
    x0b = np.ascontiguousarray(np.asarray(x0, dtype=np.float32)).astype(NPBF16)
    wt = np.ascontiguousarray(np.asarray(gcn_w, np.float32).transpose(0, 2, 1))
    bias = np.ascontiguousarray(np.asarray(gcn_b, np.float32).reshape(L, F, 1))
    w1t = np.ascontiguousarray(np.asarray(mlp_w1, np.float32).T).astype(NPBF16)
    b1 = np.ascontiguousarray(np.asarray(mlp_b1, np.float32).reshape(MLP_H, 1))
    w2t = np.ascontiguousarray(np.asarray(mlp_w2, np.float32).T)
    b2 = np.ascontiguousarray(np.asarray(mlp_b2, np.float32).reshape(1, N_CLS))
    idx = np.asarray(nodes_idx).astype(np.int64)
    # permute indices into the xg3 row layout: q*4096 + c*512 + r
    c = idx // S
    q = (idx % S) // CH
    r = idx % CH
    idxp = (q * (C * CH) + c * CH + r).astype(np.int32)

    in_maps = []
    for cc in range(C):
        at_c = np.ascontiguousarray(Ab[cc * S:(cc + 1) * S, :].T)
        in_maps.append({
            "at": at_c,
            "x0b": x0b,
            "wt": wt,
            "bias": bias,
            "w1t": w1t,
            "b1": b1,
            "w2t": w2t,
            "b2": b2,
            "idxp": idxp[cc * (N_IDX // C):(cc + 1) * (N_IDX // C)].reshape(-1, 1),
        })
    return in_maps


class _Runner:
    """Cached PJRT executor for the Bass module (axon path, 8 cores)."""

    def __init__(self, nc):
        import jax
        from jax.sharding import Mesh, PartitionSpec, NamedSharding
        from jax.experimental.shard_map import shard_map
        from concourse import bass2jax

        bass2jax.install_neuronx_cc_hook()
        self.jax = jax
        self.nc = nc

        in_names, out_names, out_avals, zero_outs = [], [], [], []
        partition_name = (nc.partition_id_tensor.name
                          if nc.partition_id_tensor else None)
        for alloc in nc.m.functions[0].allocations:
            if not isinstance(alloc, mybir.MemoryLocationSet):
                continue
            name = alloc.memorylocations[0].name
            if alloc.kind == "ExternalInput":
                if name != partition_name:
                    in_names.append(name)
            elif alloc.kind == "ExternalOutput":
                shape = tuple(alloc.tensor_shape)
                dtype = mybir.dt.np(alloc.dtype)
                out_names.append(name)
                out_avals.append(jax.core.ShapedArray(shape, dtype))
                zero_outs.append(np.zeros(shape, dtype))
        self.in_names = list(in_names)
        self.out_names = out_names
        self.out_avals = out_avals
        self.zero_outs = zero_outs
        n_params = len(in_names)
        n_outs = len(out_names)
        all_in_names = list(in_names) + list(out_names)
        if partition_name is not None:
            all_in_names.append(partition_name)
        self._meta = {
            "n_params": n_params,
            "out_avals": out_avals,
            "all_in_names": all_in_names,
            "out_names": out_names,
            "partition_name": partition_name,
        }

        def _body(*args):
            operands = list(args)
            if partition_name is not None:
                operands.append(bass2jax.partition_id_tensor())
            outs = bass2jax._bass_exec_p.bind(
                *operands,
                out_avals=tuple(out_avals),
                in_names=tuple(all_in_names),
                out_names=tuple(out_names),
                lowering_input_output_aliases=(),
                sim_require_finite=True,
                sim_require_nnan=True,
                nc=nc,
            )
            return tuple(outs)

        devices = jax.devices()[:C]
        self.mesh = Mesh(np.asarray(devices), ("core",))
        self.sharding = NamedSharding(self.mesh, PartitionSpec("core"))
        self.sharded = jax.jit(
            shard_map(_body, mesh=self.mesh,
                      in_specs=(PartitionSpec("core"),) * (n_params + n_outs),
                      out_specs=(PartitionSpec("core"),) * n_outs,
                      check_rep=False),
            donate_argnums=tuple(range(n_params, n_params + n_outs)),
            keep_unused=True)
        self.dev_inputs = None

    def put_inputs(self, in_maps):
        """Concat per-core inputs and transfer to devices once."""
        concat = [np.concatenate([np.asarray(m[n]) for m in in_maps], axis=0)
                  for n in self.in_names]
        self.dev_inputs = [self.jax.device_put(a, self.sharding) for a in concat]

    def _zeros(self):
        return [self.jax.device_put(
                    np.zeros((C * z.shape[0], *z.shape[1:]), z.dtype),
                    self.sharding)
                for z in self.zero_outs]

    def run(self):
        outs = self.sharded(*self.dev_inputs, *self._zeros())
        self.jax.block_until_ready(outs)
        return {
            name: np.asarray(outs[i]).reshape(C, *self.out_avals[i].shape)
            for i, name in enumerate(self.out_names)
        }

    def _nodonate(self):
        """Jitted single-exec without donation (safe to call repeatedly)."""
        if getattr(self, "_nodon_fn", None) is not None:
            return self._nodon_fn
        from jax.experimental.shard_map import shard_map
        from jax.sharding import PartitionSpec
        from concourse import bass2jax

        meta = self._meta

        def _body(*args):
            operands = list(args)
            if meta["partition_name"] is not None:
                operands.append(bass2jax.partition_id_tensor())
            return tuple(bass2jax._bass_exec_p.bind(
                *operands,
                out_avals=tuple(meta["out_avals"]),
                in_names=tuple(meta["all_in_names"]),
                out_names=tuple(meta["out_names"]),
                lowering_input_output_aliases=(),
                sim_require_finite=True,
                sim_require_nnan=True,
                nc=self.nc,
            ))

        n_total = meta["n_params"] + len(meta["out_names"])
        self._nodon_fn = self.jax.jit(
            shard_map(_body, mesh=self.mesh,
                      in_specs=(PartitionSpec("core"),) * n_total,
                      out_specs=(PartitionSpec("core"),) * len(meta["out_names"]),
                      check_rep=False),
            keep_unused=True)
        return self._nodon_fn

    def time_floor_diff(self, iters=5):
        """Device time ~= exec wall minus axon dispatch floor (tiny jit)."""
        import time
        zeros = self._zeros()
        tiny = self.jax.device_put(np.ones((8, 8), np.float32),
                                   self.jax.devices()[0])
        ftiny = self.jax.jit(lambda x: x + 1.0)
        self.jax.block_until_ready(ftiny(tiny))
        f = self._nodonate()
        self.jax.block_until_ready(f(*self.dev_inputs, *zeros))

        def best(fn, fargs):
            ts = []
            for _ in range(iters):
                t0 = time.perf_counter()
                self.jax.block_until_ready(fn(*fargs))
                ts.append(time.perf_counter() - t0)
            return min(ts)

        floor = best(ftiny, [tiny])
        t1 = best(f, list(self.dev_inputs) + zeros)
        return max(t1 - floor, 0.0), t1, floor

    def time_pipelined(self, k=8, iters=5):
        """Dispatch k execs without blocking, block once: if dispatch is
        async, slope over k removes the per-call round-trip latency."""
        import time
        zeros = self._zeros()
        f = self._nodonate()
        args = list(self.dev_inputs) + zeros
        self.jax.block_until_ready(f(*args))

        def run_k(kk):
            ts = []
            for _ in range(iters):
                t0 = time.perf_counter()
                outs = None
                for _ in range(kk):
                    outs = f(*args)
                self.jax.block_until_ready(outs)
                ts.append(time.perf_counter() - t0)
            return min(ts)

        t1 = run_k(1)
        tk = run_k(k)
        return (tk - t1) / (k - 1), t1, tk


def _get_runner():
    global _CACHED
    if _CACHED is None:
        nc = _build()
        _CACHED = _Runner(nc)
    return _CACHED


def kernel(A, x0, gcn_w, gcn_b, mlp_w1, mlp_b1, mlp_w2, mlp_b2, nodes_idx):
    runner = _get_runner()
    in_maps = _prep_inputs(A, x0, gcn_w, gcn_b, mlp_w1, mlp_b1, mlp_w2, mlp_b2,
                           nodes_idx)
    runner.put_inputs(in_maps)
    outs = runner.run()
    encode = outs["enc"].reshape(N_IDX, MLP_H)
    out = outs["out"].reshape(N_IDX, N_CLS)
    return encode, out


# revision 17
# speedup vs baseline: 40.8293x; 1.0313x over previous
"""Trainium2 Bass kernel for nn_MultiLevelGCN (3-layer dense GCN + MLP head).

Computation (reference):
    x = x0
    for l in range(3): x = relu((A @ x) @ W_l^T + b_l)
    h = x[nodes_idx]
    encode = relu(h @ w1^T + b1)
    out = softmax(encode @ w2^T + b2)

Sharding: 1-D row partition of A over 8 cores (2048 rows each). Each core
computes its slice of A @ x with the full x; between layers the x slices are
exchanged with chunked AllGathers (4 chunks of 512 rows per layer) so comm
overlaps the next layer's compute. The A operand is passed pre-transposed
and pre-cast to bf16 (A^T column slice, [16384, 2048] per core) so both
matmul operands have the contraction (node) dim on SBUF partitions and the
TensorE runs at 1 cycle/row with the stationary load pipelined (bf16 emits a
separate LDWEIGHTS that the PE queue pulls ahead; the fused 4-byte f32r load
is not pipelined and measures ~1.8x slower). PSUM accumulates fp32. The
small per-layer weight matmul runs f32r so z keeps near-fp32 precision.

Per layer, per core:
    z.T[f, i] = sum_j x[j, f] * A^T[j, i]   (x block stationary, A^T moving)
    xn.T[g, i] = relu(sum_f W^T[f, g] * z.T[f, i] + b[g])
    transpose xn.T -> xn (node-major bf16), store, chunked AllGather.
Head: indirect-DMA row gather of the all-gathered x3 by (permuted)
nodes_idx, MLP + softmax on 1024 rows per core.

DMA: A-stream loads are 512 KB ([512 rows, 512 cols] bf16) alternating
between the two HWDGE rings (sync=qSPDynamicHW, scalar=qActDynamicHW);
x loads are one 1 MB DMA per all-gather chunk. Measured: one ring sustains
~244 GB/s, two rings ~330 GB/s.
"""

import sys

if "/opt/trn_rl_repo" not in sys.path:
    sys.path.insert(0, "/opt/trn_rl_repo")

import ml_dtypes
import numpy as np

import concourse.bass as bass
import concourse.mybir as mybir
import concourse.tile as tile
from concourse import bacc
from concourse.masks import make_identity

N = 16384      # nodes
F = 256        # feature dim
L = 3          # gcn layers
MLP_H = 128    # mlp hidden
N_CLS = 16     # classes
N_IDX = 8192   # labeled nodes
C = 8          # cores
S = N // C     # rows per core = 2048
Q = 4          # all-gather chunks per layer
CH = S // Q    # rows per chunk = 512
P = 128        # partitions
NB = N // P    # j-blocks = 128
IC = 4         # output column chunks per core (512 each)
ICW = S // IC  # 512
NG = N // CH   # 512-row groups of the contraction dim = 32

F32 = mybir.dt.float32
F32R = mybir.dt.float32r
BF16 = mybir.dt.bfloat16
I32 = mybir.dt.int32
NPBF16 = ml_dtypes.bfloat16

AT_BUFS = 8
X_BUFS = 5
TRACE = False
LAST_EXEC_NS = None
LAST_RESULTS = None

_CACHED = None


def _build():
    nc = bacc.Bacc(trn_type="TRN2", target_bir_lowering=False, debug=False,
                   num_devices=C)

    # ---- external I/O (per core) ----
    at_d = nc.dram_tensor("at", [N, S], BF16, kind="ExternalInput")   # A^T slice, bf16
    x0_d = nc.dram_tensor("x0b", [N, F], BF16, kind="ExternalInput")  # x0, bf16
    wt_d = nc.dram_tensor("wt", [L, F, F], F32, kind="ExternalInput")  # W^T per layer [f_in, g_out]
    bias_d = nc.dram_tensor("bias", [L, F, 1], F32, kind="ExternalInput")
    w1t_d = nc.dram_tensor("w1t", [F, MLP_H], BF16, kind="ExternalInput")
    b1_d = nc.dram_tensor("b1", [MLP_H, 1], F32, kind="ExternalInput")
    w2t_d = nc.dram_tensor("w2t", [MLP_H, N_CLS], F32, kind="ExternalInput")
    b2_d = nc.dram_tensor("b2", [1, N_CLS], F32, kind="ExternalInput")
    idx_d = nc.dram_tensor("idxp", [N_IDX // C, 1], I32, kind="ExternalInput")
    enc_d = nc.dram_tensor("enc", [N_IDX // C, MLP_H], F32, kind="ExternalOutput")
    out_d = nc.dram_tensor("out", [N_IDX // C, N_CLS], F32, kind="ExternalOutput")

    # ---- internal DRAM (bf16 activations) ----
    xsl = [[nc.dram_tensor(f"xsl_{l}_{q}", [CH, F], BF16) for q in range(Q)]
           for l in range(L)]
    xg = {l: [nc.dram_tensor(f"xg_{l}_{q}", [C * CH, F], BF16, addr_space="Shared")
              for q in range(Q)]
          for l in (1, 2)}
    # Final gathered x3: one tensor so indirect DMA gathers from offset 0.
    # Row layout: q*4096 + c*512 + r  for global node j = c*2048 + q*512 + r.
    xg3_d = nc.dram_tensor("xg3", [N, F], BF16, addr_space="Shared")

    rg = [list(range(C))]

    dma_engines = [nc.sync, nc.scalar]  # the two HWDGE rings

    with tile.TileContext(nc) as tc:
        with (
            tc.tile_pool(name="xres", bufs=X_BUFS) as x_pool,
            tc.tile_pool(name="at", bufs=AT_BUFS) as at_pool,
            tc.tile_pool(name="zt", bufs=4) as zt_pool,
            tc.tile_pool(name="xnt", bufs=4) as xnt_pool,
            tc.tile_pool(name="xn", bufs=4) as xn_pool,
            tc.tile_pool(name="wconst", bufs=1) as w_pool,
            tc.tile_pool(name="head", bufs=4) as h_pool,
            tc.tile_pool(name="headc", bufs=1) as hc_pool,
            tc.tile_pool(name="sm", bufs=8) as sm_pool,
            tc.tile_pool(name="pz", bufs=4, space="PSUM") as pz_pool,
            tc.tile_pool(name="px", bufs=2, space="PSUM") as px_pool,
            tc.tile_pool(name="pt", bufs=2, space="PSUM") as pt_pool,
        ):
            # ---- constants ----
            ident_b = w_pool.tile([P, P], BF16, tag="identb")
            make_identity(nc, ident_b[:])
            ident_f = w_pool.tile([P, P], F32, tag="identf")
            make_identity(nc, ident_f[:])
            wt_sb = w_pool.tile([P, L * 2, F], F32R, tag="wt")
            for l in range(L):
                for fb in range(2):
                    nc.sync.dma_start(
                        out=wt_sb[:, l * 2 + fb, :],
                        in_=wt_d[l, fb * P:(fb + 1) * P, :].bitcast(F32R))
            bias_sb = w_pool.tile([P, L * 2], F32, tag="bias")
            for l in range(L):
                for gb in range(2):
                    nc.sync.dma_start(
                        out=bias_sb[:, l * 2 + gb:l * 2 + gb + 1],
                        in_=bias_d[l, gb * P:(gb + 1) * P, :])
            w1t_sb = w_pool.tile([P, 2, MLP_H], BF16, tag="w1t")
            for fb in range(2):
                nc.sync.dma_start(out=w1t_sb[:, fb, :],
                                  in_=w1t_d[fb * P:(fb + 1) * P, :])
            b1_sb = w_pool.tile([P, 1], F32, tag="b1")
            nc.sync.dma_start(out=b1_sb[:, :], in_=b1_d[:, :])
            w2t_sb = w_pool.tile([P, N_CLS], F32, tag="w2t")
            nc.sync.dma_start(out=w2t_sb[:, :], in_=w2t_d[:, :])
            b2_sb = w_pool.tile([1, N_CLS], F32, tag="b2")
            nc.sync.dma_start(out=b2_sb[:, :], in_=b2_d[:, :])
            ones_sb = w_pool.tile([1, P], F32, tag="ones")
            nc.gpsimd.memset(ones_sb[:, :], 1.0)

            # ---- GCN layers ----
            # Contraction rows are processed in 512-row groups. Group g covers
            # global nodes [g*512, (g+1)*512). For l>=1 availability order is
            # chunk-q major (gathered chunk q lands first); layer 0 is natural.
            for l in range(L):
                if l == 0:
                    g_list = list(range(NG))
                else:
                    g_list = [c * Q + q for q in range(Q) for c in range(C)]

                # x resident tiles: one [128, 32, 256] bf16 tile per source
                # region (layer 0: x0 quarters; l>=1: gathered chunk q).
                xt = {}
                for i, q in enumerate(range(Q)):
                    t = x_pool.tile([P, (N // Q) // P, F], BF16, tag="x", name="x")
                    if l == 0:
                        src = x0_d[q * (N // Q):(q + 1) * (N // Q), :]
                    else:
                        src = xg[l][q][:, :]
                    # DRAM rows (s*128 + p, f) -> SBUF (p, s, f). On the
                    # SWDGE queue: an AG-gated load on a HWDGE ring would
                    # block the ring FIFO and stall next-layer A prefetch.
                    nc.gpsimd.dma_start(
                        out=t[:], in_=src.rearrange("(s p) f -> p s f", p=P))
                    xt[q] = t

                def x_slot(j0):
                    # (tile, slot) holding global nodes [j0, j0+128)
                    if l == 0:
                        return xt[j0 // (N // Q)], (j0 % (N // Q)) // P
                    c, rem = divmod(j0, S)
                    q, r = divmod(rem, CH)
                    return xt[q], c * (CH // P) + r // P

                # Two output super-chunks of 1024 cols; each A^T tile is 1 MB
                # ([512 rows, 1024 cols] bf16, 2 KB segments) consumed by four
                # PSUM accumulation groups (2 sub-chunks x 2 feature blocks).
                for icp in range(IC // 2):
                    pz = [[pz_pool.tile([P, ICW], F32, tag="pz", name="pz")
                           for _ in range(2)] for _ in range(2)]  # [sub][fb]
                    for gi, g in enumerate(g_list):
                        at_t = at_pool.tile([P, CH // P, 2 * ICW], BF16,
                                            tag="at", name="at")
                        dma_engines[gi % 2].dma_start(
                            out=at_t[:],
                            in_=at_d[g * CH:(g + 1) * CH,
                                     icp * 2 * ICW:(icp + 1) * 2 * ICW]
                            .rearrange("(r p) c -> p r c", p=P))
                        for rb in range(CH // P):
                            xtile, slot = x_slot(g * CH + rb * P)
                            for sub in range(2):
                                for fb in range(2):
                                    nc.tensor.matmul(
                                        out=pz[sub][fb][:],
                                        lhsT=xtile[:, slot, fb * P:(fb + 1) * P],
                                        rhs=at_t[:, rb, sub * ICW:(sub + 1) * ICW],
                                        start=(gi == 0 and rb == 0),
                                        stop=(gi == NG - 1 and rb == CH // P - 1))

                    for sub in range(2):
                        ic = icp * 2 + sub
                        zt = []
                        for fb in range(2):
                            z = zt_pool.tile([P, ICW], F32R, tag="zt")
                            nc.vector.tensor_copy(out=z[:], in_=pz[sub][fb][:])
                            zt.append(z)

                        xnt = []
                        for gb in range(2):
                            px = px_pool.tile([P, ICW], F32, tag="px")
                            for fb in range(2):
                                nc.tensor.matmul(
                                    out=px[:],
                                    lhsT=wt_sb[:, l * 2 + fb, gb * P:(gb + 1) * P],
                                    rhs=zt[fb][:],
                                    start=(fb == 0), stop=(fb == 1))
                            xt_out = xnt_pool.tile([P, ICW], BF16, tag="xnt")
                            nc.scalar.activation(
                                out=xt_out[:], in_=px[:],
                                func=mybir.ActivationFunctionType.Relu,
                                bias=bias_sb[:, l * 2 + gb:l * 2 + gb + 1])
                            xnt.append(xt_out)

                        for isub in range(ICW // P):
                            xn = xn_pool.tile([P, F], BF16, tag="xn")
                            for gb in range(2):
                                ptt = pt_pool.tile([P, P], BF16, tag="pt")
                                nc.tensor.transpose(
                                    out=ptt[:],
                                    in_=xnt[gb][:, isub * P:(isub + 1) * P],
                                    identity=ident_b[:])
                                nc.vector.tensor_copy(
                                    out=xn[:, gb * P:(gb + 1) * P], in_=ptt[:])
                            nc.gpsimd.dma_start(
                                out=xsl[l][ic][isub * P:(isub + 1) * P, :],
                                in_=xn[:])

                        outs = (xg[l + 1][ic][:, :] if l < L - 1
                                else xg3_d[ic * C * CH:(ic + 1) * C * CH, :])
                        nc.gpsimd.collective_compute(
                            "AllGather", mybir.AluOpType.bypass,
                            replica_groups=rg,
                            ins=[xsl[l][ic][:, :]],
                            outs=[outs])

            # ---- head: gather + MLP + softmax ----
            n_ch = (N_IDX // C) // P  # 8 chunks of 128 indices
            ht = [hc_pool.tile([P, n_ch * P], BF16, tag=f"ht{fb}", name=f"ht{fb}")
                  for fb in range(2)]
            for ch in range(n_ch):
                idx_t = h_pool.tile([P, 1], I32, tag="idx")
                nc.sync.dma_start(out=idx_t[:], in_=idx_d[ch * P:(ch + 1) * P, :])
                h = h_pool.tile([P, F], BF16, tag="h")
                nc.gpsimd.indirect_dma_start(
                    out=h[:], out_offset=None,
                    in_=xg3_d[:, :],
                    in_offset=bass.IndirectOffsetOnAxis(ap=idx_t[:, :1], axis=0))
                for fb in range(2):
                    ptt = pt_pool.tile([P, P], BF16, tag="pt")
                    nc.tensor.transpose(out=ptt[:], in_=h[:, fb * P:(fb + 1) * P],
                                        identity=ident_b[:])
                    nc.vector.tensor_copy(out=ht[fb][:, ch * P:(ch + 1) * P],
                                          in_=ptt[:])

            et = hc_pool.tile([P, n_ch * P], F32, tag="et")  # encode.T [m, i2]
            for i2c in range(2):
                pe = px_pool.tile([P, ICW], F32, tag="px")
                for fb in range(2):
                    nc.tensor.matmul(
                        out=pe[:],
                        lhsT=w1t_sb[:, fb, :],
                        rhs=ht[fb][:, i2c * ICW:(i2c + 1) * ICW],
                        start=(fb == 0), stop=(fb == 1))
                nc.scalar.activation(
                    out=et[:, i2c * ICW:(i2c + 1) * ICW], in_=pe[:],
                    func=mybir.ActivationFunctionType.Relu,
                    bias=b1_sb[:, 0:1])

            for ch in range(n_ch):
                ptt = pt_pool.tile([P, P], F32, tag="pt", name="ptf")
                nc.tensor.transpose(out=ptt[:], in_=et[:, ch * P:(ch + 1) * P],
                                    identity=ident_f[:])
                enc_t = h_pool.tile([P, MLP_H], F32, tag="enc")
                nc.vector.tensor_copy(out=enc_t[:], in_=ptt[:])
                nc.sync.dma_start(out=enc_d[ch * P:(ch + 1) * P, :], in_=enc_t[:])

                pl = pt_pool.tile([P, N_CLS], F32, tag="pt", name="ptf")
                nc.tensor.matmul(out=pl[:], lhsT=ones_sb[:, :], rhs=b2_sb[:, :],
                                 start=True, stop=False, skip_group_check=True)
                nc.tensor.matmul(out=pl[:], lhsT=et[:, ch * P:(ch + 1) * P],
                                 rhs=w2t_sb[:, :],
                                 start=False, stop=True, skip_group_check=True)
                nmax = sm_pool.tile([P, 1], F32, tag="nmax")
                nc.vector.tensor_reduce(out=nmax[:], in_=pl[:],
                                        axis=mybir.AxisListType.X,
                                        op=mybir.AluOpType.max, negate=True)
                ex = sm_pool.tile([P, N_CLS], F32, tag="ex")
                ssum = sm_pool.tile([P, 1], F32, tag="ssum")
                nc.scalar.activation(out=ex[:], in_=pl[:],
                                     func=mybir.ActivationFunctionType.Exp,
                                     bias=nmax[:, 0:1],
                                     accum_out=ssum[:, 0:1])
                rs = sm_pool.tile([P, 1], F32, tag="rs")
                nc.vector.reciprocal(out=rs[:], in_=ssum[:])
                ot = sm_pool.tile([P, N_CLS], F32, tag="ot")
                nc.scalar.activation(out=ot[:], in_=ex[:],
                                     func=mybir.ActivationFunctionType.Copy,
                                     scale=rs[:, 0:1])
                nc.sync.dma_start(out=out_d[ch * P:(ch + 1) * P, :], in_=ot[:])

    nc.compile()
    return nc


def _prep_inputs(A, x0, gcn_w, gcn_b, mlp_w1, mlp_b1, mlp_w2, mlp_b2, nodes_idx):
    A = np.asarray(A, dtype=np.float32)
    Ab = A.astype(NPBF16)
    x0b = np.ascontiguousarray(np.asarray(x0, dtype=np.float32)).astype(NPBF16)
    wt = np.ascontiguousarray(np.asarray(gcn_w, np.float32).transpose(0, 2, 1))
    bias = np.ascontiguousarray(np.asarray(gcn_b, np.float32).reshape(L, F, 1))
    w1t = np.ascontiguousarray(np.asarray(mlp_w1, np.float32).T).astype(NPBF16)
    b1 = np.ascontiguousarray(np.asarray(mlp_b1, np.float32).reshape(MLP_H, 1))
    w2t = np.ascontiguousarray(np.asarray(mlp_w2, np.float32).T)
    b2 = np.ascontiguousarray(np.asarray(mlp_b2, np.float32).reshape(1, N_CLS))
    idx = np.asarray(nodes_idx).astype(np.int64)
    # permute indices into the xg3 row layout: q*4096 + c*512 + r
    c = idx // S
    q = (idx % S) // CH
    r = idx % CH
    idxp = (q * (C * CH) + c * CH + r).astype(np.int32)

    in_maps = []
    for cc in range(C):
        at_c = np.ascontiguousarray(Ab[cc * S:(cc + 1) * S, :].T)
        in_maps.append({
            "at": at_c,
            "x0b": x0b,
            "wt": wt,
            "bias": bias,
            "w1t": w1t,
            "b1": b1,
            "w2t": w2t,
            "b2": b2,
            "idxp": idxp[cc * (N_IDX // C):(cc + 1) * (N_IDX // C)].reshape(-1, 1),
        })
    return in_maps


class _Runner:
    """Cached PJRT executor for the Bass module (axon path, 8 cores)."""

    def __init__(self, nc):
        import jax
        from jax.sharding import Mesh, PartitionSpec, NamedSharding
        from jax.experimental.shard_map import shard_map
        from concourse import bass2jax

        bass2jax.install_neuronx_cc_hook()
        self.jax = jax
        self.nc = nc

        in_names, out_names, out_avals, zero_outs = [], [], [], []
        partition_name = (nc.partition_id_tensor.name
                          if nc.partition_id_tensor else None)
        for alloc in nc.m.functions[0].allocations:
            if not isinstance(alloc, mybir.MemoryLocationSet):
                continue
            name = alloc.memorylocations[0].name
            if alloc.kind == "ExternalInput":
                if name != partition_name:
                    in_names.append(name)
            elif alloc.kind == "ExternalOutput":
                shape = tuple(alloc.tensor_shape)
                dtype = mybir.dt.np(alloc.dtype)
                out_names.append(name)
                out_avals.append(jax.core.ShapedArray(shape, dtype))
                zero_outs.append(np.zeros(shape, dtype))
        self.in_names = list(in_names)
        self.out_names = out_names
        self.out_avals = out_avals
        self.zero_outs = zero_outs
        n_params = len(in_names)
        n_outs = len(out_names)
        all_in_names = list(in_names) + list(out_names)
        if partition_name is not None:
            all_in_names.append(partition_name)
        self._meta = {
            "n_params": n_params,
            "out_avals": out_avals,
            "all_in_names": all_in_names,
            "out_names": out_names,
            "partition_name": partition_name,
        }

        def _body(*args):
            operands = list(args)
            if partition_name is not None:
                operands.append(bass2jax.partition_id_tensor())
            outs = bass2jax._bass_exec_p.bind(
                *operands,
                out_avals=tuple(out_avals),
                in_names=tuple(all_in_names),
                out_names=tuple(out_names),
                lowering_input_output_aliases=(),
                sim_require_finite=True,
                sim_require_nnan=True,
                nc=nc,
            )
            return tuple(outs)

        devices = jax.devices()[:C]
        self.mesh = Mesh(np.asarray(devices), ("core",))
        self.sharding = NamedSharding(self.mesh, PartitionSpec("core"))
        self.sharded = jax.jit(
            shard_map(_body, mesh=self.mesh,
                      in_specs=(PartitionSpec("core"),) * (n_params + n_outs),
                      out_specs=(PartitionSpec("core"),) * n_outs,
                      check_rep=False),
            donate_argnums=tuple(range(n_params, n_params + n_outs)),
            keep_unused=True)
        self.dev_inputs = None

    def put_inputs(self, in_maps):
        """Concat per-core inputs and transfer to devices once."""
        concat = [np.concatenate([np.asarray(m[n]) for m in in_maps], axis=0)
                  for n in self.in_names]
        self.dev_inputs = [self.jax.device_put(a, self.sharding) for a in concat]

    def _zeros(self):
        return [self.jax.device_put(
                    np.zeros((C * z.shape[0], *z.shape[1:]), z.dtype),
                    self.sharding)
                for z in self.zero_outs]

    def run(self):
        outs = self.sharded(*self.dev_inputs, *self._zeros())
        self.jax.block_until_ready(outs)
        return {
            name: np.asarray(outs[i]).reshape(C, *self.out_avals[i].shape)
            for i, name in enumerate(self.out_names)
        }

    def _nodonate(self):
        """Jitted single-exec without donation (safe to call repeatedly)."""
        if getattr(self, "_nodon_fn", None) is not None:
            return self._nodon_fn
        from jax.experimental.shard_map import shard_map
        from jax.sharding import PartitionSpec
        from concourse import bass2jax

        meta = self._meta

        def _body(*args):
            operands = list(args)
            if meta["partition_name"] is not None:
                operands.append(bass2jax.partition_id_tensor())
            return tuple(bass2jax._bass_exec_p.bind(
                *operands,
                out_avals=tuple(meta["out_avals"]),
                in_names=tuple(meta["all_in_names"]),
                out_names=tuple(meta["out_names"]),
                lowering_input_output_aliases=(),
                sim_require_finite=True,
                sim_require_nnan=True,
                nc=self.nc,
            ))

        n_total = meta["n_params"] + len(meta["out_names"])
        self._nodon_fn = self.jax.jit(
            shard_map(_body, mesh=self.mesh,
                      in_specs=(PartitionSpec("core"),) * n_total,
                      out_specs=(PartitionSpec("core"),) * len(meta["out_names"]),
                      check_rep=False),
            keep_unused=True)
        return self._nodon_fn

    def time_floor_diff(self, iters=5):
        """Device time ~= exec wall minus axon dispatch floor (tiny jit)."""
        import time
        zeros = self._zeros()
        tiny = self.jax.device_put(np.ones((8, 8), np.float32),
                                   self.jax.devices()[0])
        ftiny = self.jax.jit(lambda x: x + 1.0)
        self.jax.block_until_ready(ftiny(tiny))
        f = self._nodonate()
        self.jax.block_until_ready(f(*self.dev_inputs, *zeros))

        def best(fn, fargs):
            ts = []
            for _ in range(iters):
                t0 = time.perf_counter()
                self.jax.block_until_ready(fn(*fargs))
                ts.append(time.perf_counter() - t0)
            return min(ts)

        floor = best(ftiny, [tiny])
        t1 = best(f, list(self.dev_inputs) + zeros)
        return max(t1 - floor, 0.0), t1, floor

    def time_pipelined(self, k=8, iters=5):
        """Dispatch k execs without blocking, block once: if dispatch is
        async, slope over k removes the per-call round-trip latency."""
        import time
        zeros = self._zeros()
        f = self._nodonate()
        args = list(self.dev_inputs) + zeros
        self.jax.block_until_ready(f(*args))

        def run_k(kk):
            ts = []
            for _ in range(iters):
                t0 = time.perf_counter()
                outs = None
                for _ in range(kk):
                    outs = f(*args)
                self.jax.block_until_ready(outs)
                ts.append(time.perf_counter() - t0)
            return min(ts)

        t1 = run_k(1)
        tk = run_k(k)
        return (tk - t1) / (k - 1), t1, tk


def _get_runner():
    global _CACHED
    if _CACHED is None:
        nc = _build()
        _CACHED = _Runner(nc)
    return _CACHED


def kernel(A, x0, gcn_w, gcn_b, mlp_w1, mlp_b1, mlp_w2, mlp_b2, nodes_idx):
    runner = _get_runner()
    in_maps = _prep_inputs(A, x0, gcn_w, gcn_b, mlp_w1, mlp_b1, mlp_w2, mlp_b2,
                           nodes_idx)
    runner.put_inputs(in_maps)
    outs = runner.run()
    encode = outs["enc"].reshape(N_IDX, MLP_H)
    out = outs["out"].reshape(N_IDX, N_CLS)
    return encode, out
